# revision 39
# baseline (speedup 1.0000x reference)
"""Segment-sum (scatter-add) kernel for Trainium2, SPMD over 8 NeuronCores.

Problem: out[n, :] = sum over edges e with X_node[e] == n of H[e, :]
  H [E=800000, 64] f32, X_node [E] int64, node_num N=50000 -> out [N, 64] f32.

Strategy (canonical class masks - no per-edge mask bytes)
---------------------------------------------------------
Nodes are sorted by edge count and grouped into "windows" of nn same-class
nodes, where a class is a count value c (counts < 8 pad up to 8, leftovers
lift into the next class so every class's window count is divisible by 8).
A window has a fixed capacity cap(c) in {256..1536} slots chosen to
minimize slack, nn = min(64, cap // c) nodes, and node j's edges occupy
slots [j*c, j*c + count_j) (rest zero).  The one-hot mask for a window is
therefore a CANONICAL stripe pattern depending only on the class - the ~21
class masks (~1.3 MB fp8) are DMA'd once (over the SWDGE ring, parallel to
the HW-DGE load path) and stay resident in SBUF, so the edge stream
carries only data: a 2-way fp8(e4m3) cascade hi + mid == H to ~2^-10
relative (mid = fp8(H - hi) unscaled, e4m3 subnormals carry small
residuals), 128 B/edge-slot at ~1.9% slot padding.

The global window sequence (class runs divisible by 8) is dealt round-robin
to the 8 cores, so every core runs ONE identical program with the same
hardcoded class schedule.  Pairs of 128-slot blocks pack into 256-slot
super-blocks in the DoubleRow interleave layout: per partition
[hi|mid(e0) | hi|mid(e1)], with the resident class mask as lhsT.

Device kernel per core:
  PE:  psum[64, 128] += stripe_mask.T @ [hi|mid] as ONE DoubleRow fp8
       matmul per super-block; SB_w-super accumulation group per window;
       G_PS=16 windows per 4-bank PSUM tile, 2 tiles ping-pong.
  ACT: tmp = psum_mid (PSUM->SBUF copy, batched over the group)
  DVE: out = psum_hi + tmp -> [64, D] f16 (fp16 halves store traffic; one
       extra ~2^-11 rounding, far inside the 2e-2 error budget)
  DMA: sync ring streams packed chunks (ramped sizes both ends: quick
       start, short drain); gpsimd SWDGE ring does stores (mostly parallel
       to the HW-DGE load path, so stores barely steal load bandwidth).
Host gathers window rows into out (pure layout, no arithmetic).
"""

import os

import numpy as np
import ml_dtypes

FP8 = np.dtype(ml_dtypes.float8_e4m3)

N_CORES = 8
P = 128
D = 64
WN = 64    # mask width: max nodes per window
G_PS = 16  # windows per PSUM tile / fold batch
G_TL = 2   # windows per fold batch for the tail groups
N_TL = 4   # number of tail windows folded in small batches
CH = 32    # steady-state super-blocks (256 slots each) per DMA chunk
CMIN = 8   # minimum class: smaller counts pad up


def _chunk_plan(S):
    """Chunk sizes (in super-blocks): ramp up 4,4,8,16 so the first matmul
    only waits for a small chunk, steady CH, then ramp down 16,8,4 so the
    compute tail after the final load is short."""
    up = [4, 4, 8, 16]
    down = [8, 4, 2, 2]
    if S <= sum(up) + sum(down):
        sizes = []
        t = 0
        while t < S:
            s = min(8, S - t)
            sizes.append(s)
            t += s
        return sizes
    mid = S - sum(up) - sum(down)
    sizes = list(up)
    while mid > 0:
        s = min(CH, mid)
        sizes.append(s)
        mid -= s
    sizes.extend(down)
    return sizes


def _group_plan(W):
    """Fold-group sizes (in windows): G_PS steady, but the last N_TL
    windows fold in batches of G_TL so the post-last-load tail is short."""
    tail = min(W, N_TL)
    head = W - tail
    gs = []
    w = 0
    while w < head:
        g = min(G_PS, head - w)
        gs.append(g)
        w += g
    while w < W:
        g = min(G_TL, W - w)
        gs.append(g)
        w += g
    return gs


# ----------------------------------------------------------------- planning
def _nn_cap(c, k=None):
    """Best (nn, cap) for class c holding k nodes, minimizing per-core READ
    bytes: padded H slots (128 B each) + the resident class-mask tile (read
    once; 2*WN*P B per super-block).  Stores are writes and ride free, so
    they don't enter the cost.  Caps start at 512 (>= 2 super-blocks per
    window) so a 16-window fold group always carries enough PE work to hide
    the ACT+DVE fold chain."""
    best = None
    for cap in (512, 768, 1024, 1280, 1536):
        nn = min(WN, cap // c)
        if nn == 0:
            continue
        if k:
            w = -(-(-(-k // nn)) // 8) * 8
            cost = (w * cap * 16 + (cap // 256) * 2 * WN * P, -cap)
        else:
            cost = (round((cap - nn * c) / cap, 6), -cap)
        if best is None or cost < best[0]:
            best = (cost, cap, nn)
    return best[2], best[1]


def _plan(X, N):
    """Global class schedule + per-core window node lists.

    Returns (sched, wins_core, cum, order) where
      sched: list of (c, nn, cap) per GLOBAL window index (identical class
             sequence for every core once dealt round-robin),
      wins_core[core]: list of node-id arrays (one per local window),
      cum/order: edge sort fanout (cum[n]..cum[n+1] slices order -> edge ids
             of node n).
    """
    order = np.argsort(X, kind="stable")
    counts = np.bincount(X, minlength=N)
    cum = np.zeros(N + 1, dtype=np.int64)
    np.cumsum(counts, out=cum[1:])

    cmax = int(counts.max())
    cmax = max(cmax, CMIN)
    # pool[c] = node ids assigned to class c (count <= c)
    pools = {c: [] for c in range(CMIN, cmax + 2)}
    csort = np.argsort(counts, kind="stable")  # ascending count
    for n in csort:
        pools[max(CMIN, int(counts[n]))].append(int(n))

    runs = []  # class runs: (cap, [(c, nn, cap, node_list), ...])
    for c in range(CMIN, cmax + 1):
        nn, cap = _nn_cap(c, len(pools[c]))
        k = len(pools[c])
        w8 = (k // (nn * 8)) * 8
        used = w8 * nn
        if k - used:
            pools[c + 1] = pools[c][used:] + pools[c + 1]  # lift leftovers
        if w8:
            runs.append(
                (cap, [(c, nn, cap, pools[c][w * nn : (w + 1) * nn])
                       for w in range(w8)])
            )
    # top-level leftovers: final class, padded to a multiple of 8 windows
    c = cmax + 1
    k = len(pools[c])
    nn, cap = _nn_cap(c, k)
    w8 = -(-max(k, 1) // nn)
    w8 = -(-w8 // 8) * 8
    runs.append(
        (cap, [(c, nn, cap, pools[c][w * nn : (w + 1) * nn])
               for w in range(w8)])
    )
    # interleave big- and small-capacity runs so every 16-window fold group
    # carries enough PE work (super-blocks) to hide the fold chain; keep the
    # smallest-cap run for the very END so the post-last-load tail (final
    # matmuls + folds) is as short as possible
    runs.sort(key=lambda r: r[0])
    last = runs.pop(0) if len(runs) > 1 else None
    lo, hi = 0, len(runs) - 1
    gw = []
    while lo <= hi:
        gw.extend(runs[hi][1])
        hi -= 1
        if lo <= hi:
            gw.extend(runs[lo][1])
            lo += 1
    if last is not None:
        gw.extend(last[1])
    # pad global count to a multiple of 16 (8 cores x 2 partition blocks)
    # with empty windows
    nn0, cap0 = _nn_cap(CMIN)
    while len(gw) % (8 * 2):
        gw.append((CMIN, nn0, cap0, []))

    sched = [(c, nn, cap) for c, nn, cap, _ in gw[:: N_CORES]]
    wins_core = [
        [nodes for _, _, _, nodes in gw[cr::N_CORES]] for cr in range(N_CORES)
    ]
    return sched, wins_core, cum, order


def _build_masks(sched):
    """Canonical stripe masks, one per distinct class, DoubleRow-interleaved.

    Returns (mask_blob [P, MW] fp8, class_off dict c -> column offset).
    Per class the layout is SB super-blocks x [P, 2, WN]."""
    classes = []
    seen = set()
    for c, nn, cap in sched:
        if c not in seen:
            seen.add(c)
            classes.append((c, nn, cap))
    cols = []
    class_off = {}
    off = 0
    for c, nn, cap in classes:
        slots = np.arange(cap)
        node = slots // c  # stripe: slot s -> node s//c
        msk = (node[:, None] == np.arange(WN)[None, :]) & (node[:, None] < nn)
        msk = msk.astype(FP8)  # [cap, WN]
        sb = cap // 256
        m = msk.reshape(sb, 2, P, WN).transpose(0, 2, 1, 3)  # [sb, P, 2, WN]
        cols.append(m.reshape(sb, P, 2 * WN))
        class_off[c] = off
        off += sb * 2 * WN
    blob = np.concatenate(cols, axis=0)  # [SBtot, P, 2*WN]
    blob = np.ascontiguousarray(blob.transpose(1, 0, 2).reshape(P, off))
    return blob, class_off


def _build_core_inputs(H32, cum, order, wins, sched):
    """Packed [hi|mid] DoubleRow edge stream for one core."""
    S = sum(cap // 256 for _, _, cap in sched)
    Hg = np.zeros((S * 2 * P, D), dtype=np.float32)
    s0 = 0  # slot cursor
    for (c, nn, cap), nodes in zip(sched, wins, strict=True):
        for j, n in enumerate(nodes):
            e0 = int(cum[n])
            e1 = int(cum[n + 1])
            base = s0 + j * c
            Hg[base : base + (e1 - e0)] = H32[order[e0:e1]]
        s0 += cap
    # 2-way fp8 cascade: hi + mid == H to ~2^-10 relative (mid unscaled,
    # e4m3 subnormals carry the small residuals; accumulates fold-free).
    hi = Hg.astype(FP8)
    mid = (Hg - hi.astype(np.float32)).astype(FP8)
    H2 = np.concatenate([hi, mid], axis=1)  # [S*2*P, 2D]
    pk = H2.reshape(S, 2, P, 2 * D).transpose(2, 0, 1, 3)  # [P, S, 2, 2D]
    return np.ascontiguousarray(pk.reshape(P, S * 4 * D))


# ------------------------------------------------------------- device kernel
def _build_program(sched, class_off, MW, mw_a):
    import concourse.bacc as bacc
    import concourse.tile as tile
    import concourse.mybir as mybir

    nc = bacc.Bacc("TRN2", target_bir_lowering=False, debug=False)
    fp8 = mybir.dt.float8e4
    f32 = mybir.dt.float32
    f16 = mybir.dt.float16

    PKW = 4 * D  # packed fp8 super-row: [hi|mid(e0) | hi|mid(e1)]
    W = len(sched)
    S = sum(cap // 256 for _, _, cap in sched)
    with tile.TileContext(nc) as tc:
        with tc.tile_pool(name="dram", bufs=1, space="DRAM") as dram:
            pkt = dram.tile([P, S * PKW], fp8, kind="ExternalInput")
            mskd = dram.tile([P, MW], fp8, kind="ExternalInput")
            # fp16 output: halves store traffic; one extra rounding at
            # ~2^-11 relative, far inside the error budget
            odev = dram.tile([WN, W * D], f16, kind="ExternalOutput")

            with tc.tile_pool(name="mbuf", bufs=1) as mpool, \
                 tc.tile_pool(name="hbuf", bufs=6) as hpool, \
                 tc.tile_pool(name="psum", bufs=2, space="PSUM") as pspool, \
                 tc.tile_pool(name="outb", bufs=6) as opool:

                msk = mpool.tile([P, MW], fp8)
                # both mask parts ride the SWDGE ring (parallel to the
                # HW-DGE load path): the early part is first in the queue
                # and lands before the first chunk; the bulk is emitted
                # later in program order so it doesn't steal DMA engines
                # during the load ramp
                nc.gpsimd.dma_start(out=msk[:, :mw_a], in_=mskd[:, :mw_a])

                chunk_starts = {}
                t_acc = 0
                for s in _chunk_plan(S):
                    chunk_starts[t_acc] = s
                    t_acc += s

                groups = _group_plan(W)

                pk = None
                t0 = 0
                t = 0
                ps = None
                gi = 0      # group index
                g0 = 0      # first window of current group
                for w, (c, nn, cap) in enumerate(sched):
                    if w == 8 and mw_a < MW:
                        nc.gpsimd.dma_start(
                            out=msk[:, mw_a:], in_=mskd[:, mw_a:]
                        )
                    g = w - g0
                    ng = groups[gi]
                    if g == 0:
                        ps = pspool.tile([WN, ng, 2 * D], f32)
                    SB = cap // 256
                    moff = class_off[c]
                    for b in range(SB):
                        if t in chunk_starts:
                            ch = chunk_starts[t]
                            t0 = t
                            pk = hpool.tile([P, CH, PKW], fp8, tag="h")
                            nc.sync.dma_start(
                                out=pk[:, :ch, :],
                                in_=pkt[:, t * PKW : (t + ch) * PKW].rearrange(
                                    "p (c d) -> p c d", c=ch
                                ),
                            )
                        rel = t - t0
                        nc.tensor.matmul(
                            out=ps[:, g, :],
                            lhsT=msk[
                                :, moff + b * 2 * WN : moff + (b + 1) * 2 * WN
                            ].rearrange("p (k m) -> p k m", k=2),
                            rhs=pk[:, rel, :].rearrange("p (k n) -> p k n", k=2),
                            start=(b == 0),
                            stop=(b == SB - 1),
                            perf_mode=mybir.MatmulPerfMode.DoubleRow,
                        )
                        t += 1
                    if g == ng - 1:
                        # fold: out = psum_hi + psum_mid (no scaling).  ACT
                        # stages the mid half through SBUF (DVE can't read
                        # two PSUM operands in one op).
                        tm = opool.tile([WN, ng, D], f32, tag="t")
                        nc.scalar.copy(out=tm[:, :, :], in_=ps[:, :, D : 2 * D])
                        ot = opool.tile([WN, ng, D], f16, tag="o")
                        nc.vector.tensor_tensor(
                            out=ot[:, :, :],
                            in0=ps[:, :, 0:D],
                            in1=tm[:, :, :],
                            op=mybir.AluOpType.add,
                        )
                        # SWDGE ring: parallel to the HW-DGE load path
                        nc.gpsimd.dma_start(
                            out=odev[:, g0 * D : (w + 1) * D].rearrange(
                                "n (g f) -> n g f", g=ng
                            ),
                            in_=ot[:, :, :],
                        )
                        gi += 1
                        g0 = w + 1
    nc.compile()
    return nc, pkt, mskd, odev


# --------------------------------------------------------------------- main
def kernel(H, X_node, node_num):
    from concourse import bass_utils

    H32 = np.asarray(H, dtype=np.float32)
    X = np.asarray(X_node).astype(np.int64)
    N = int(node_num)
    E = X.shape[0]
    assert H32.shape == (E, D)

    sched, wins_core, cum, order = _plan(X, N)
    W = len(sched)

    blob, class_off = _build_masks(sched)
    MW = blob.shape[1]
    early = {c for c, _, _ in sched[:24]}
    mw_a = max(
        class_off[c] + (cap // 256) * 2 * WN
        for c, _, cap in sched
        if c in early
    )
    if mw_a >= MW - 2 * WN:
        mw_a = MW
    nc, pkt, mskd, odev = _build_program(sched, class_off, MW, mw_a)
    in_maps = []
    for cr in range(N_CORES):
        pkt_np = _build_core_inputs(H32, cum, order, wins_core[cr], sched)
        in_maps.append({pkt.name: pkt_np, mskd.name: blob})

    trace = bool(int(os.environ.get("SEGSUM_TRACE", "0")))
    res = bass_utils.run_bass_kernel_spmd(
        nc, in_maps, core_ids=list(range(N_CORES)), trace=trace
    )
    if trace:
        kernel.last_exec_time_ns = res.exec_time_ns
        kernel.last_mean_exec_time_ns = res.mean_exec_time_ns
        kernel.last_trace = (
            res.instructions_and_trace[1] if res.instructions_and_trace else None
        )

    out = np.zeros((N, D), dtype=np.float32)
    for cr in range(N_CORES):
        ot = res.results[cr][odev.name].reshape(WN, W, D).astype(np.float32)
        for w, nodes in enumerate(wins_core[cr]):
            if nodes:
                out[np.asarray(nodes), :] = ot[: len(nodes), w, :]
    return out
